# revision 1
# baseline (speedup 1.0000x reference)
"""Causal self-attention (B=2, T=2048, C=1024, H=16) on 8 TRN2 NeuronCores.

Sharding: tensor-parallel over heads — core c owns heads {2c, 2c+1} for both
batches (qkv_w column slice, o_w row slice). Each core computes a partial
o_proj output; the host sums the 8 partials and adds o_b.

Kernel math (per core), all matmuls in float32r (TF32-like, full PE rate):
  xT = transpose(x_b)                      (PE transpose-mode, 128x128 blocks)
  qT/kT/vT = W_slice^T @ x^T + bias        (weights stationary, N=512 moving)
  per (batch, head):  S^T[j,i] = kT^T qT   (K=64; heads packed at partition 0/64)
  P^T = exp(S^T/8)                         (ACT, PSUM->SBUF, [128,1024] groups)
  causal: lower j-blocks skipped, diagonal blocks masked by a 128x128 tri mask
  O_aug^T[d+1, i] = V_aug^T P^T            (V augmented with a ones column ->
                                            row 64 of the accumulator is the
                                            softmax denominator, zero cost)
  attT = O^T * (1/denominator)             (DVE; denom broadcast via GpSimd)
  Y[t, e] = attT^T @ o_w_slice             (attT stationary, N=512 moving)

Emission is unit-interleaved so PE stays dense while ACT runs the exps:
  [xpose+qkv b0] [attn b0 || xpose+qkv b1] [attn b1 || oproj b0] [oproj b1]
"""

import numpy as np

B = 2
T = 2048
C = 1024
H = 16
DH = 64
NCORES = 8
HL = 2                      # heads per core
HCOLS = HL * DH             # 128
TB = T // 128               # 16 t-blocks per batch
KB = C // 128               # 8 k-blocks
NCH = T // 512              # 4 i-chunks per batch
TH = T // 2

CFG = {"b1": 2, "b2": 2, "pv": 2, "pt": 4}

_nc_cache = None


def _interleave(primary, filler):
    """Emit primary units with filler units woven in (filler spread evenly)."""
    np_, nf = len(primary), len(filler)
    fi = 0
    for i, u in enumerate(primary):
        u()
        want = int(round((i + 1) * nf / max(np_, 1)))
        while fi < want:
            filler[fi]()
            fi += 1
    while fi < nf:
        filler[fi]()
        fi += 1


def build_bass(dbg=False, loop_n=0):
    import concourse.bass as bass
    import concourse.bacc as bacc
    import concourse.tile as tile
    import concourse.mybir as mybir

    F32 = mybir.dt.float32
    F32R = mybir.dt.float32r
    Exp = mybir.ActivationFunctionType.Exp

    nc = bacc.Bacc("TRN2", target_bir_lowering=False, debug=False)

    x_d = nc.dram_tensor("x", [B, T, C], F32R, kind="ExternalInput")
    w_d = nc.dram_tensor("w", [C, 3 * HCOLS], F32R, kind="ExternalInput")
    bias_d = nc.dram_tensor("bqkv", [HCOLS, 3], F32, kind="ExternalInput")
    ow_d = nc.dram_tensor("ow", [HCOLS, C], F32R, kind="ExternalInput")
    ident_d = nc.dram_tensor("ident", [128, 128], F32R, kind="ExternalInput")
    tri_d = nc.dram_tensor("tri", [128, 128], F32R, kind="ExternalInput")
    ones_d = nc.dram_tensor("ones", [128, TB], F32R, kind="ExternalInput")
    y_d = nc.dram_tensor("y", [B * T, C], F32, kind="ExternalOutput")
    if dbg:
        qT_dbg = nc.dram_tensor("qT_dbg", [128, B * T], F32, kind="ExternalOutput")
        kT_dbg = nc.dram_tensor("kT_dbg", [128, B * T], F32, kind="ExternalOutput")
        attT_dbg = nc.dram_tensor("attT_dbg", [128, B * T], F32, kind="ExternalOutput")

    with tile.TileContext(nc) as tc:
        with (
            tc.tile_pool(name="const", bufs=1) as constp,
            tc.tile_pool(name="xnat", bufs=4) as xnatp,
            tc.tile_pool(name="xT", bufs=1) as xtp,
            tc.tile_pool(name="qkv", bufs=1) as qkvp,
            tc.tile_pool(name="vaug", bufs=2) as vaugp,
            tc.tile_pool(name="pT", bufs=4) as ptp,
            tc.tile_pool(name="att", bufs=1) as attp,
            tc.tile_pool(name="recip", bufs=2) as recipp,
            tc.tile_pool(name="yout", bufs=3) as youtp,
            tc.tile_pool(name="ps", bufs=1, space="PSUM") as ps,
        ):
            # ---- constants / weights ----
            w_sb = constp.tile([128, KB * 3 * 128], F32R)      # [k, (kb, m*128)]
            ow_sb = constp.tile([128, C], F32R)
            bias_sb = constp.tile([HCOLS, 3], F32)
            ident_sb = constp.tile([128, 128], F32R)
            tri_sb = constp.tile([128, 128], F32R)
            nc.sync.dma_start(
                w_sb[:].rearrange("p (kb d) -> p kb d", d=3 * 128),
                w_d.rearrange("(kb p) d -> p kb d", p=128),
            )
            nc.sync.dma_start(ow_sb[:], ow_d[:])
            nc.sync.dma_start(bias_sb[:], bias_d[:])
            nc.sync.dma_start(ident_sb[:], ident_d[:])
            nc.sync.dma_start(tri_sb[:], tri_d[:])
            w3 = w_sb[:].rearrange("p (kb d) -> p kb d", d=3 * 128)

            xT = xtp.tile([128, KB * TH], F32R)               # [k, (kb, t)] half batch
            qT = qkvp.tile([128, B * T], F32R, name="qT")     # [2 heads*64, (b, t)]
            kT = qkvp.tile([128, B * T], F32R, name="kT")
            vT = qkvp.tile([128, T], F32R, name="vT")         # current batch
            attT = attp.tile([128, B * T], F32R)

            def xpose_unit(b, half, tbl):
                """Transpose one 128-row block of x into xT (8 kb blocks)."""
                tb = (TB // 2) * half + tbl

                def run():
                    xnat = xnatp.tile([128, C], F32R, name="xnat")
                    nc.sync.dma_start(xnat[:], x_d[b, tb * 128:(tb + 1) * 128, :])
                    pt = ps.tile([128, 1024], F32R, name="psx", tag="b2",
                                 bufs=CFG["b2"])
                    for kb in range(KB):
                        nc.tensor.transpose(
                            pt[:, kb * 128:(kb + 1) * 128],
                            xnat[:, kb * 128:(kb + 1) * 128],
                            ident_sb[:],
                        )
                    dst = xT[:].rearrange("p (kb t) -> p kb t", t=TH)[
                        :, :, tbl * 128:(tbl + 1) * 128]
                    src = pt[:].rearrange("p (i t) -> p i t", t=128)
                    nc.vector.tensor_copy(dst, src)
                return run

            def qkv_unit(b, half, m, tcl):
                """One 512-wide t-chunk of q/k/v^T projection."""
                tc_ = (NCH // 2) * half + tcl
                dstT = (qT, kT, vT)[m]

                def run():
                    pt = ps.tile([128, 512], F32, name="psqkv", tag="b1",
                                 bufs=CFG["b1"])
                    for kb in range(KB):
                        nc.tensor.matmul(
                            pt[:],
                            w3[:, kb, m * 128:(m + 1) * 128],
                            xT[:, kb * TH + tcl * 512: kb * TH + tcl * 512 + 512],
                            start=(kb == 0),
                            stop=(kb == KB - 1),
                        )
                    vdst = dstT[:, tc_ * 512:tc_ * 512 + 512] if m == 2 \
                        else dstT[:, b * T + tc_ * 512: b * T + tc_ * 512 + 512]
                    nc.vector.tensor_scalar_add(vdst, pt[:], bias_sb[:, m:m + 1])
                return run

            def xpose_qkv_units(b):
                units = []
                for half in range(2):
                    for tbl in range(TB // 2):
                        units.append(xpose_unit(b, half, tbl))
                    for m in range(3):
                        for tcl in range(NCH // 2):
                            units.append(qkv_unit(b, half, m, tcl))
                return units

            def vaug_units(b, vaugs):
                """Build v_aug tiles for both heads of batch b (2 units)."""
                units = []
                for h in range(HL):
                    def run(h=h):
                        va = vaugp.tile([128, TB * 65], F32R, name=f"vaug{h}",
                                        tag=f"va{h}")
                        nc.sync.dma_start(
                            va[:].rearrange("p (tb d) -> p tb d", d=65)[:, :, 64:65],
                            ones_d.rearrange("p (tb o) -> p tb o", o=1))
                        pt = ps.tile([128, 1024], F32R, name="psva", tag="b2",
                                     bufs=CFG["b2"])
                        for tb in range(TB):
                            nc.tensor.transpose(
                                pt[:, tb * 64:(tb + 1) * 64],
                                vT[h * 64:(h + 1) * 64, tb * 128:(tb + 1) * 128],
                                ident_sb[h * 64:(h + 1) * 64, h * 64:(h + 1) * 64],
                            )
                        dst = va[:].rearrange("p (tb d) -> p tb d", d=65)[:, :, 0:64]
                        src = pt[:].rearrange("p (i d) -> p i d", d=64)
                        nc.vector.tensor_copy(dst, src)
                        vaugs[h] = va
                    units.append(run)
                return units

            def attention_units(b, vaugs):
                """Unit per j-block pair (scores+exp+PV both heads); the last
                unit of each i-chunk also normalizes into attT."""
                units = []
                for ic in range(NCH):
                    i0 = 512 * ic
                    n_jb = 4 * (ic + 1)
                    pv = [None, None]

                    for g in range(n_jb // 2):
                        def run(g=g, ic=ic, i0=i0, n_jb=n_jb, pv=pv):
                            if g == 0:
                                for h in range(HL):
                                    pv[h] = ps.tile([128, 512], F32,
                                                    name=f"pspv{h}", tag="pv",
                                                    bufs=CFG["pv"])
                            sc = [ps.tile([128, 1024], F32, name=f"pssc{h}",
                                          tag="b2", bufs=CFG["b2"])
                                  for h in range(HL)]
                            for u in range(2):
                                jb = 2 * g + u
                                j0 = 128 * jb
                                for h in range(HL):
                                    nc.tensor.matmul(
                                        sc[h][:, u * 512:(u + 1) * 512],
                                        kT[h * 64:(h + 1) * 64,
                                           b * T + j0: b * T + j0 + 128],
                                        qT[h * 64:(h + 1) * 64,
                                           b * T + i0: b * T + i0 + 512],
                                        start=True, stop=True,
                                    )
                            pts = []
                            for h in range(HL):
                                pt_sb = ptp.tile([128, 1024], F32R, name=f"pt{h}",
                                                 tag=f"pt{h}", bufs=CFG["pt"])
                                nc.scalar.activation(
                                    pt_sb[:], sc[h][:], Exp,
                                    scale=float(1.0 / np.sqrt(DH)))
                                pts.append(pt_sb)
                            for u in range(2):
                                jb = 2 * g + u
                                o = 128 * jb - i0
                                lo = max(o, 0)
                                for h in range(HL):
                                    if o >= 0:
                                        seg = pts[h][:, u * 512 + o:
                                                     u * 512 + o + 128]
                                        nc.vector.tensor_tensor(
                                            seg, seg, tri_sb[:],
                                            mybir.AluOpType.mult)
                                    nc.tensor.matmul(
                                        pv[h][0:65, lo:512],
                                        vaugs[h][:, jb * 65: jb * 65 + 65],
                                        pts[h][:, u * 512 + lo:(u + 1) * 512],
                                        start=(jb == 0),
                                        stop=(jb == n_jb - 1),
                                    )
                            if 2 * g + 1 == n_jb - 1:   # chunk done: normalize
                                for h in range(HL):
                                    rrow = recipp.tile([1, 512], F32,
                                                       name="rrow", tag="rr")
                                    rbc = recipp.tile([64, 512], F32,
                                                      name="rbc", tag="rb")
                                    nc.vector.reciprocal(rrow[:], pv[h][64:65, :])
                                    nc.gpsimd.partition_broadcast(rbc[:], rrow[:])
                                    nc.vector.tensor_tensor(
                                        attT[h * 64:(h + 1) * 64,
                                             b * T + i0: b * T + i0 + 512],
                                        pv[h][0:64, :], rbc[:],
                                        mybir.AluOpType.mult)
                        units.append(run)
                return units

            def oproj_units(b, act_share=False):
                units = []
                for tb in range(TB):
                    def run(tb=tb):
                        yo = youtp.tile([128, C], F32, name="yo")
                        for ec in range(2):
                            pt = ps.tile([128, 512], F32, name="psy", tag="b1",
                                         bufs=CFG["b1"])
                            nc.tensor.matmul(
                                pt[:],
                                attT[:, b * T + tb * 128: b * T + (tb + 1) * 128],
                                ow_sb[:, ec * 512:(ec + 1) * 512],
                                start=True, stop=True,
                            )
                            dst = yo[:, ec * 512:(ec + 1) * 512]
                            if ec == 0 or not act_share:
                                nc.vector.tensor_copy(dst, pt[:])
                            else:
                                nc.scalar.activation(
                                    dst, pt[:], mybir.ActivationFunctionType.Copy)
                        nc.sync.dma_start(
                            y_d[b * T + tb * 128: b * T + (tb + 1) * 128, :], yo[:])
                    units.append(run)
                return units

            def _schedule():
                va0, va1 = [None, None], [None, None]
                for u in xpose_qkv_units(0):
                    u()
                for u in vaug_units(0, va0):
                    u()
                _interleave(attention_units(0, va0),
                            xpose_qkv_units(1) + vaug_units(1, va1))
                if dbg:
                    nc.sync.dma_start(qT_dbg[:], qT[:].bitcast(F32))
                    nc.sync.dma_start(kT_dbg[:], kT[:].bitcast(F32))
                a1 = attention_units(1, va1)
                o1 = oproj_units(1, act_share=True)
                seq = []
                oi = 0
                for i, u in enumerate(a1):
                    seq.append(u)
                    if i in (1, 5, 11, 19):   # i-chunk of attn(1) complete
                        seq.extend(o1[oi:oi + 4])
                        oi += 4
                _interleave(seq, oproj_units(0))
                if dbg:
                    nc.sync.dma_start(attT_dbg[:], attT[:].bitcast(F32))

            # ---- schedule ----
            if loop_n:
                with tc.For_i(0, loop_n, 1):
                    _schedule()
            else:
                _schedule()

    nc.compile()
    return nc


def _prep_inputs(x, qkv_w, qkv_b, o_w):
    """Per-core input maps (head sharding)."""
    ident = np.eye(128, dtype=np.float32)
    tri = np.triu(np.ones((128, 128), dtype=np.float32))
    x = np.ascontiguousarray(np.asarray(x, dtype=np.float32))
    qkv_w = np.asarray(qkv_w, dtype=np.float32)
    qkv_b = np.asarray(qkv_b, dtype=np.float32)
    o_w = np.asarray(o_w, dtype=np.float32)
    in_maps = []
    for c in range(NCORES):
        lo = c * HCOLS
        w_c = np.concatenate(
            [qkv_w[:, lo:lo + HCOLS],
             qkv_w[:, C + lo:C + lo + HCOLS],
             qkv_w[:, 2 * C + lo:2 * C + lo + HCOLS]], axis=1)
        b_c = np.stack(
            [qkv_b[lo:lo + HCOLS],
             qkv_b[C + lo:C + lo + HCOLS],
             qkv_b[2 * C + lo:2 * C + lo + HCOLS]], axis=1)
        ow_c = o_w[lo:lo + HCOLS, :]
        in_maps.append({
            "x": x,
            "w": np.ascontiguousarray(w_c),
            "bqkv": np.ascontiguousarray(b_c),
            "ow": np.ascontiguousarray(ow_c),
            "ident": ident,
            "tri": tri,
            "ones": np.ones((128, TB), dtype=np.float32),
        })
    return in_maps


def kernel(x, qkv_w, qkv_b, o_w, o_b):
    global _nc_cache
    from concourse import bass_utils
    if _nc_cache is None:
        _nc_cache = build_bass()
    nc = _nc_cache
    in_maps = _prep_inputs(x, qkv_w, qkv_b, o_w)
    res = bass_utils.run_bass_kernel_spmd(nc, in_maps, core_ids=list(range(NCORES)))
    y = np.zeros((B * T, C), dtype=np.float64)
    for c in range(NCORES):
        y += res.results[c]["y"].astype(np.float64)
    y = (y + np.asarray(o_b, dtype=np.float64)[None, :]).astype(np.float32)
    return y.reshape(B, T, C)



# revision 2
# speedup vs baseline: 1.0004x; 1.0004x over previous
"""Causal self-attention (B=2, T=2048, C=1024, H=16) on 8 TRN2 NeuronCores.

Sharding: tensor-parallel over heads - core c owns heads {2c, 2c+1} for both
batches (qkv_w column slice, o_w row slice). Each core computes a partial
o_proj output; the host sums the 8 partials (fp64) and adds o_b.

Design (all fp16 operands, fp32 psum accumulation):
  - x^T is pre-transposed ON HOST and fed as an fp16 input: no PE transposes,
    no psum->sbuf x copies, half the DMA bytes. Weights/biases are pre-sliced
    and cast on host as well.
  - q^T/k^T = W^T x^T (w stationary, xT moving 512 wide); the bias add rides
    the mandatory psum->sbuf copy (DVE tensor_scalar).
  - v is produced in NATURAL [t, dh] layout (stationary xT block, moving w_v)
    directly into vaug tiles carrying a ones column (softmax denominator
    accumulates inside the PV matmul for free).
  - S^T[j,i] per (batch, j-block, i-chunk) with causal trimming; one fused
    exp per chunk on ACT (scale folded in, both heads via a 2-segment AP)
    into a per-batch P^T arena; diagonal blocks masked by a 0/1 triangle
    on GpSimd (sbuf-only).
  - PV: vaug stationary [j,65], P^T moving -> unnormalized attT psum
    [65, 512]; row 64 is the denominator. reciprocal (DVE) + partition
    broadcast (GpSimd) + normalize-multiply into attT fp16.
  - oproj: attT stationary, ow moving; psum->sbuf copies balanced across
    DVE/ACT; y partials stream out per 512-column half as fp16.
  Schedule: software-pipelined attention units (PV lags S/exp by one unit so
  the ACT exp latency hides behind the next S matmuls); attention(b0) starts
  during qkv(b0); qkv(b1) fills attention(b0); oproj(b0) fills
  attention(b1); oproj(b1) chunks are spliced in as their attT columns
  normalize, with an ACT/DVE-parallel drain for the last blocks.
"""

import numpy as np

B = 2
T = 2048
C = 1024
H = 16
DH = 64
NCORES = 8
HL = 2                      # heads per core
HCOLS = HL * DH             # 128
KB = C // 128               # 8 contraction blocks
TB = T // 128               # 16 t-blocks per batch
NCH = T // 512              # 4 i-chunks per batch

# per-jb P^T extents (columns per head) and offsets in the per-batch arena
EXTS = [T - 128 * jb for jb in range(TB)]
OFFS = np.cumsum([0] + [2 * e for e in EXTS]).tolist()
PT_COLS = OFFS[-1]          # 2 * 17408 = 34816

CFG = {
    "norm_eng": ["dve"],                  # rotation for normalize-mult (psum: no pool)
    "yo_eng": ["dve"],
    "mask_eng": ["pool"],                 # P^T masks are sbuf-only: pool OK
    "sq_bufs": 2,
    "pv_bufs": 2,
    "b1_bufs": 2,
    "xt_bufs": 3,
}

_nc_cache = None


def _interleave(primary, filler):
    """Emit primary units with filler units woven in (filler spread evenly)."""
    np_, nf = len(primary), len(filler)
    fi = 0
    for i, u in enumerate(primary):
        u()
        want = int(round((i + 1) * nf / max(np_, 1)))
        while fi < want:
            filler[fi]()
            fi += 1
    while fi < nf:
        filler[fi]()
        fi += 1


def build_bass(dbg=False):
    import concourse.bass as bass
    import concourse.bacc as bacc
    import concourse.tile as tile
    import concourse.mybir as mybir

    F32 = mybir.dt.float32
    F16 = mybir.dt.float16
    Exp = mybir.ActivationFunctionType.Exp
    Mult = mybir.AluOpType.mult
    Add = mybir.AluOpType.add

    nc = bacc.Bacc("TRN2", target_bir_lowering=False, debug=False)

    xt_d = nc.dram_tensor("xt", [C, B * T], F16, kind="ExternalInput")
    wqk_d = nc.dram_tensor("wqk", [C, 2 * HCOLS], F16, kind="ExternalInput")
    wv_d = nc.dram_tensor("wv", [C, HCOLS], F16, kind="ExternalInput")
    bqk_d = nc.dram_tensor("bqk", [HCOLS, 2], F32, kind="ExternalInput")
    bv_d = nc.dram_tensor("bv", [128, 512], F32, kind="ExternalInput")
    ow_d = nc.dram_tensor("ow", [HCOLS, C], F16, kind="ExternalInput")
    tri2_d = nc.dram_tensor("tri2", [128, 256], F16, kind="ExternalInput")
    y_d = nc.dram_tensor("y", [B * T, C], F16, kind="ExternalOutput")

    scale = float(1.0 / np.sqrt(DH))

    with tile.TileContext(nc) as tc:
        with (
            tc.tile_pool(name="const", bufs=1) as constp,
            tc.tile_pool(name="xt", bufs=CFG["xt_bufs"]) as xtp,
            tc.tile_pool(name="qk", bufs=2) as qkp,
            tc.tile_pool(name="vaug", bufs=2) as vaugp,
            tc.tile_pool(name="pt", bufs=1) as ptp,
            tc.tile_pool(name="att", bufs=2) as attp,
            tc.tile_pool(name="recip", bufs=4) as recipp,
            tc.tile_pool(name="yout", bufs=3) as youtp,
            tc.tile_pool(name="ps", bufs=1, space="PSUM") as ps,
        ):
            # ---- constants / weights ----
            wqk_sb = constp.tile([128, KB * 2 * 128], F16)    # [k, (kb, m*128)]
            wv_sb = constp.tile([128, KB * 128], F16)         # [k, (kb, dh2)]
            ow_sb = constp.tile([128, C], F16)
            bqk_sb = constp.tile([HCOLS, 2], F32)
            bv_sb = constp.tile([128, 512], F32)
            tri2_sb = constp.tile([128, 256], F16)

            wqk3 = wqk_sb[:].rearrange("p (kb d) -> p kb d", d=2 * 128)
            wv3 = wv_sb[:].rearrange("p (kb d) -> p kb d", d=128)

            # ---- per-batch state (tiles acquired per batch) ----
            st = [dict() for _ in range(B)]   # xt, qT, kT, vaug, pt, attT

            def startup_unit():
                """wqk first, then x^T(b0,H0) in 512-token halves so the first
                qk chunk completes as early as possible."""
                nc.sync.dma_start(
                    wqk_sb[:].rearrange("p (kb d) -> p kb d", d=2 * 128),
                    wqk_d.rearrange("(kb p) d -> p kb d", p=128),
                )
                xts = xtp.tile([128, KB * 1024], F16, name="xt00", tag="xt")
                st[0]["xt0"] = xts
                dst = xts[:].rearrange("p (kb t) -> p kb t", t=1024)
                src = xt_d.rearrange("(kb p) t -> p kb t", p=128)[:, :, 0:1024]
                nc.sync.dma_start(dst[:, :, 0:512], src[:, :, 0:512])
                nc.sync.dma_start(bqk_sb[:], bqk_d[:])
                nc.sync.dma_start(
                    wv_sb[:].rearrange("p (kb d) -> p kb d", d=128),
                    wv_d.rearrange("(kb p) d -> p kb d", p=128),
                )
                nc.sync.dma_start(bv_sb[:], bv_d[:])
                nc.sync.dma_start(dst[:, :, 512:1024], src[:, :, 512:1024])
                nc.sync.dma_start(ow_sb[:], ow_d[:])
                nc.sync.dma_start(tri2_sb[:], tri2_d[:])

            def xt_dma_unit(b, Hh):
                """DMA x^T for half-batch Hh into a fresh tile."""
                def run():
                    xts = xtp.tile([128, KB * 1024], F16, name=f"xt{b}{Hh}",
                                   tag="xt")
                    st[b][f"xt{Hh}"] = xts
                    dst = xts[:].rearrange("p (kb t) -> p kb t", t=1024)
                    src = xt_d.rearrange("(kb p) t -> p kb t", p=128)[
                        :, :, b * T + Hh * 1024: b * T + Hh * 1024 + 1024]
                    nc.sync.dma_start(dst, src)
                return run

            def alloc_batch_unit(b):
                def run():
                    st[b]["qT"] = qkp.tile([128, T], F16, name=f"qT{b}", tag="qT")
                    st[b]["kT"] = qkp.tile([128, T], F16, name=f"kT{b}", tag="kT")
                    va = vaugp.tile([128, TB * 130], F16, name=f"va{b}", tag="va")
                    st[b]["va"] = va
                    st[b]["pt"] = ptp.tile([128, PT_COLS], F16, name=f"pt{b}",
                                           tag="pt")
                    st[b]["attT"] = attp.tile([128, T], F16, name=f"attT{b}",
                                              tag="attT")
                    # ones columns of vaug (denominator accumulators)
                    ones_view = va[:].rearrange("p (g d) -> p g d", d=65)[:, :, 64:65]
                    nc.vector.memset(ones_view, 1.0)
                return run

            def qk_unit(b, Hh, m, tcl):
                """One 512-token chunk of q^T or k^T (m: 0=q, 1=k)."""
                def run():
                    xts = st[b][f"xt{Hh}"]
                    xv = xts[:].rearrange("p (kb t) -> p kb t", t=1024)
                    pt = ps.tile([128, 512], F32, name="psqk", tag="b1",
                                 bufs=CFG["b1_bufs"])
                    for kb in range(KB):
                        nc.tensor.matmul(
                            pt[:],
                            wqk3[:, kb, m * 128:(m + 1) * 128],
                            xv[:, kb, tcl * 512:tcl * 512 + 512],
                            start=(kb == 0),
                            stop=(kb == KB - 1),
                        )
                    dstT = st[b]["qT"] if m == 0 else st[b]["kT"]
                    t0 = Hh * 1024 + tcl * 512
                    nc.vector.tensor_scalar_add(
                        dstT[:, t0:t0 + 512], pt[:], bqk_sb[:, m:m + 1])
                return run

            def v_unit(b, Hh, g):
                """v for 4 t-blocks (512 tokens), natural layout into vaug."""
                def run():
                    xts = st[b][f"xt{Hh}"]
                    xv = xts[:].rearrange("p (kb t) -> p kb t", t=1024)
                    va = st[b]["va"]
                    pt = ps.tile([128, 512], F32, name="psv", tag="b1",
                                 bufs=CFG["b1_bufs"])
                    for tbl in range(4):
                        tloc = g * 512 + tbl * 128
                        for kb in range(KB):
                            nc.tensor.matmul(
                                pt[:, tbl * 128:(tbl + 1) * 128],
                                xv[:, kb, tloc:tloc + 128],
                                wv3[:, kb, :],
                                start=(kb == 0),
                                stop=(kb == KB - 1),
                            )
                    # psum [t(128), 4tb x (2h x 64dh)] -> vaug [tb, h*65 .. +64]
                    tb0 = Hh * 8 + g * 4
                    dst = va[:].rearrange("p (tb c) -> p tb c", c=130)[
                        :, tb0:tb0 + 4, :].rearrange("p tb (h c) -> p tb h c",
                                                     h=2)[:, :, :, 0:64]
                    src = pt[:].rearrange("p (tb h c) -> p tb h c", tb=4, h=2)
                    nc.vector.tensor_tensor(
                        dst, src,
                        bv_sb[:].rearrange("p (tb h c) -> p tb h c", tb=4, h=2),
                        Add)
                return run

            def qkv_units(b):
                units = []
                for Hh in range(2):
                    for tcl in range(2):
                        units.append(qk_unit(b, Hh, 0, tcl))
                        units.append(qk_unit(b, Hh, 1, tcl))
                        units.append(v_unit(b, Hh, tcl))
                return units

            # ---- attention ----
            eng_tt = {"dve": nc.vector, "pool": nc.gpsimd}

            def s_exp_part(b, ic, jb):
                """S matmuls + exp (+ diag mask) for (i-chunk ic, j-block jb)."""
                i0 = 512 * ic
                lo = max(0, 128 * jb - i0)          # >0 only on diagonal chunk
                ext = EXTS[jb]
                off = OFFS[jb]
                rel = i0 + lo - 128 * jb            # chunk start within extent
                n = 512 - lo

                def run():
                    qT, kT = st[b]["qT"], st[b]["kT"]
                    ptt = st[b]["pt"]
                    sc = ps.tile([128, 1024], F32, name="pssc", tag="sq",
                                 bufs=CFG["sq_bufs"])
                    for h in range(HL):
                        nc.tensor.matmul(
                            sc[:, h * 512 + lo:(h + 1) * 512],
                            kT[h * 64:(h + 1) * 64, 128 * jb:128 * jb + 128],
                            qT[h * 64:(h + 1) * 64, i0 + lo:i0 + 512],
                            start=True, stop=True,
                        )
                    # exp -> P^T arena (2-segment strided AP covers both heads)
                    src = sc[:].rearrange("p (h x) -> p h x", h=2)[:, :, lo:512]
                    dst = ptt[:, off:off + 2 * ext].rearrange(
                        "p (h x) -> p h x", h=2)[:, :, rel:rel + n]
                    nc.scalar.activation(dst, src, Exp, scale=scale)
                    # mask the diagonal block right after its exp
                    if rel == 0:
                        mdst = ptt[:, off:off + 2 * ext].rearrange(
                            "p (h x) -> p h x", h=2)[:, :, 0:128]
                        meng = eng_tt[CFG["mask_eng"][jb % len(CFG["mask_eng"])]]
                        meng.tensor_tensor(
                            mdst, mdst,
                            tri2_sb[:].rearrange("p (h x) -> p h x", h=2),
                            Mult)
                return run

            def pv_part(b, ic, jb):
                i0 = 512 * ic
                lo = max(0, 128 * jb - i0)
                ext = EXTS[jb]
                off = OFFS[jb]
                rel = i0 + lo - 128 * jb
                n = 512 - lo
                n_jb = 4 * (ic + 1)

                def run():
                    ptt = st[b]["pt"]
                    va = st[b]["va"]
                    if jb == 0:
                        for h in range(HL):
                            st[b][f"pv{h}"] = ps.tile(
                                [128, 512], F32, name=f"pspv{h}", tag="pv",
                                bufs=CFG["pv_bufs"])
                    for h in range(HL):
                        nc.tensor.matmul(
                            st[b][f"pv{h}"][0:65, lo:512],
                            va[:, jb * 130 + h * 65:jb * 130 + h * 65 + 65],
                            ptt[:, off + h * ext + rel:off + h * ext + rel + n],
                            start=(jb == 0),
                            stop=(jb == n_jb - 1),
                        )
                return run

            def norm_part(b, ic, tail=False):
                i0 = 512 * ic

                def run():
                    attT = st[b]["attT"]
                    for h in range(HL):
                        pv = st[b][f"pv{h}"]
                        rrow = recipp.tile([1, 512], F32, name="rrow", tag="rr")
                        rbc = recipp.tile([64, 512], F32, name="rbc", tag="rb")
                        if b == 0:
                            # free the pv psum bank early for the next chunk
                            au = recipp.tile([65, 512], F32, name="attU",
                                             tag="au", bufs=2)
                            nc.vector.tensor_copy(au[:], pv[0:65, :])
                            src = au
                        else:
                            src = pv
                        nc.vector.reciprocal(rrow[:], src[64:65, :])
                        nc.gpsimd.partition_broadcast(rbc[:], rrow[:])
                        ngrp = 4 if tail else 1
                        w = 512 // ngrp
                        for g in range(ngrp):
                            nc.vector.tensor_tensor(
                                attT[h * 64:(h + 1) * 64,
                                     i0 + g * w:i0 + (g + 1) * w],
                                src[0:64, g * w:(g + 1) * w],
                                rbc[:, g * w:(g + 1) * w], Mult)
                return run

            def attention_units(b, after_chunk=None):
                """Software-pipelined: unit k = S/exp(order[k]) + PV(order[k-1]).
                after_chunk[ic] (optional) -> list of units spliced right after
                the unit that closes chunk ic (PV last + norm).
                Returns the unit list."""
                order = [(ic, jb) for ic in range(NCH)
                         for jb in range(4 * (ic + 1))]
                units = []

                def mk(parts):
                    def run():
                        for p in parts:
                            p()
                    return run

                prev = None
                pending = []
                for (ic, jb) in order:
                    parts = [s_exp_part(b, ic, jb)]
                    if prev is not None:
                        parts.append(pv_part(b, *prev))
                        if prev[1] == 4 * (prev[0] + 1) - 1:
                            parts.append(norm_part(b, prev[0], tail=(b == 1 and prev[0] == NCH - 1)))
                    units.append(mk(parts))
                    if pending:
                        units.extend(pending)
                        pending = []
                    if prev is not None and prev[1] == 4 * (prev[0] + 1) - 1 \
                            and after_chunk:
                        pending = list(after_chunk.get(prev[0], []))
                    prev = (ic, jb)
                if pending:
                    units.extend(pending)
                    pending = []
                units.append(mk([pv_part(b, *prev), norm_part(b, prev[0], tail=(b == 1 and prev[0] == NCH - 1))]))
                if after_chunk:
                    units.extend(after_chunk.get(NCH - 1, []))
                return units

            # ---- output projection ----
            def oproj_unit(b, tb):
                def run():
                    attT = st[b]["attT"]
                    yo = youtp.tile([128, C], F16, name="yo", tag="yo")
                    for ec in range(2):
                        pt = ps.tile([128, 512], F32, name="psy", tag="b1",
                                     bufs=CFG["b1_bufs"])
                        nc.tensor.matmul(
                            pt[:],
                            attT[:, tb * 128:(tb + 1) * 128],
                            ow_sb[:, ec * 512:(ec + 1) * 512],
                            start=True, stop=True,
                        )
                        if b == 1 and tb >= 12:
                            # tail: split copies across ACT+DVE so the last
                            # blocks drain in parallel
                            eng = "act" if ec == 0 else "dve"
                        else:
                            eng = CFG["yo_eng"][(b * TB + tb * 2 + ec)
                                                % len(CFG["yo_eng"])]
                        dst = yo[:, ec * 512:(ec + 1) * 512]
                        if eng == "dve":
                            nc.vector.tensor_copy(dst, pt[:])
                        elif eng == "act":
                            nc.scalar.copy(dst, pt[:])
                        else:
                            nc.gpsimd.tensor_copy(dst, pt[:])
                        nc.sync.dma_start(
                            y_d[b * T + tb * 128:b * T + (tb + 1) * 128,
                                ec * 512:(ec + 1) * 512], dst)
                return run

            def oproj_units(b):
                return [oproj_unit(b, tb) for tb in range(TB)]

            # ---- schedule ----
            startup_unit()
            alloc_batch_unit(0)()
            qk0 = qkv_units(0)
            a0 = attention_units(0)      # 41 pipelined units
            # b0: attention chunks woven into qkv as soon as deps exist
            seq0 = (qk0[0:3] + a0[0:4] + [xt_dma_unit(0, 1)] + qk0[3:6]
                    + a0[4:12] + qk0[6:9] + a0[12:24] + qk0[9:12] + a0[24:41])
            q1 = qkv_units(1)
            fill1 = ([xt_dma_unit(1, 0), alloc_batch_unit(1)] + q1[0:6]
                     + [xt_dma_unit(1, 1)] + q1[6:12])
            # keep early PE stream unobstructed; weave b1 qkv into the tail
            for u in seq0[:12]:
                u()
            _interleave(seq0[12:], fill1)
            o0 = oproj_units(0)
            o1 = oproj_units(1)
            a1 = attention_units(1, after_chunk={
                0: o1[0:4], 1: o1[4:8], 2: o1[8:12], 3: o1[12:16]})
            _interleave(a1, o0)

    nc.compile()
    return nc


def _prep_inputs(x, qkv_w, qkv_b, o_w):
    """Per-core input maps (head sharding), with host-side transpose/casts."""
    x = np.asarray(x, dtype=np.float32)
    qkv_w = np.asarray(qkv_w, dtype=np.float32)
    qkv_b = np.asarray(qkv_b, dtype=np.float32)
    o_w = np.asarray(o_w, dtype=np.float32)

    xt = np.ascontiguousarray(
        x.reshape(B * T, C).T.astype(np.float16))          # [C, B*T]
    tri = np.triu(np.ones((128, 128), dtype=np.float16))
    tri2 = np.ascontiguousarray(np.concatenate([tri, tri], axis=1))

    in_maps = []
    for c in range(NCORES):
        lo = c * HCOLS
        wq = qkv_w[:, lo:lo + HCOLS]
        wk = qkv_w[:, C + lo:C + lo + HCOLS]
        wv = qkv_w[:, 2 * C + lo:2 * C + lo + HCOLS]
        bq = qkv_b[lo:lo + HCOLS]
        bk = qkv_b[C + lo:C + lo + HCOLS]
        bv = qkv_b[2 * C + lo:2 * C + lo + HCOLS]
        # v bias replicated for the 4-t-block psum layout [tb(4) x dh2(128)]
        bv_rep = np.tile(bv[None, :], (128, 4)).astype(np.float32)
        in_maps.append({
            "xt": xt,
            "wqk": np.ascontiguousarray(
                np.concatenate([wq, wk], axis=1).astype(np.float16)),
            "wv": np.ascontiguousarray(wv.astype(np.float16)),
            "bqk": np.ascontiguousarray(
                np.stack([bq, bk], axis=1).astype(np.float32)),
            "bv": np.ascontiguousarray(bv_rep),
            "ow": np.ascontiguousarray(
                o_w[lo:lo + HCOLS, :].astype(np.float16)),
            "tri2": tri2,
        })
    return in_maps


def kernel(x, qkv_w, qkv_b, o_w, o_b):
    global _nc_cache
    from concourse import bass_utils
    if _nc_cache is None:
        _nc_cache = build_bass()
    nc = _nc_cache
    in_maps = _prep_inputs(x, qkv_w, qkv_b, o_w)
    res = bass_utils.run_bass_kernel_spmd(nc, in_maps, core_ids=list(range(NCORES)))
    y = np.zeros((B * T, C), dtype=np.float64)
    for c in range(NCORES):
        y += res.results[c]["y"].astype(np.float64)
    y = (y + np.asarray(o_b, dtype=np.float64)[None, :]).astype(np.float32)
    return y.reshape(B, T, C)


# revision 4
# speedup vs baseline: 1.0315x; 1.0311x over previous
"""Causal self-attention (B=2, T=2048, C=1024, H=16) on 8 TRN2 NeuronCores.

Sharding: tensor-parallel over heads - core c owns heads {2c, 2c+1} for both
batches (qkv_w column slice, o_w row slice). Each core computes a partial
o_proj output; the host sums the 8 partials (fp64) and adds o_b.

Design (all fp16 operands, fp32 psum accumulation):
  - x^T is pre-transposed ON HOST and fed as an fp16 input: no PE transposes,
    no psum->sbuf x copies, half the DMA bytes. Weights/biases are pre-sliced
    and cast on host as well.
  - q^T/k^T = W^T x^T (w stationary, xT moving 512 wide); the bias add rides
    the mandatory psum->sbuf copy (DVE tensor_scalar).
  - v is produced in NATURAL [t, dh] layout (stationary xT block, moving w_v)
    directly into vaug tiles carrying a ones column (softmax denominator
    accumulates inside the PV matmul for free).
  - S^T[j,i] per (batch, j-block, i-chunk) with causal trimming; one fused
    exp per chunk on ACT (scale folded in, both heads via a 2-segment AP)
    into a per-batch P^T arena; diagonal blocks masked by a 0/1 triangle
    on GpSimd (sbuf-only).
  - PV: vaug stationary [j,65], P^T moving -> unnormalized attT psum
    [65, 512]; row 64 is the denominator. reciprocal (DVE) + partition
    broadcast (GpSimd) + normalize-multiply into attT fp16.
  - oproj: attT stationary, ow moving; psum->sbuf copies balanced across
    DVE/ACT; y partials stream out per 512-column half as fp16.
  Schedule: software-pipelined attention units (PV lags S/exp by one unit so
  the ACT exp latency hides behind the next S matmuls); attention(b0) starts
  during qkv(b0); qkv(b1) fills attention(b0); oproj(b0) fills
  attention(b1); oproj(b1) chunks are spliced in as their attT columns
  normalize, with an ACT/DVE-parallel drain for the last blocks.
"""

import numpy as np

B = 2
T = 2048
C = 1024
H = 16
DH = 64
NCORES = 8
HL = 2                      # heads per core
HCOLS = HL * DH             # 128
KB = C // 128               # 8 contraction blocks
TB = T // 128               # 16 t-blocks per batch
NCH = T // 512              # 4 i-chunks per batch

# per-jb P^T extents (columns per head) and offsets in the per-batch arena
EXTS = [T - 128 * jb for jb in range(TB)]
OFFS = np.cumsum([0] + [2 * e for e in EXTS]).tolist()
PT_COLS = OFFS[-1]          # 2 * 17408 = 34816

CFG = {
    "norm_eng": ["dve"],                  # rotation for normalize-mult (psum: no pool)
    "yo_eng": ["dve"],
    "mask_eng": ["pool"],                 # P^T masks are sbuf-only: pool OK
    "sq_bufs": 2,
    "pv_bufs": 2,
    "b1_bufs": 2,
    "xt_bufs": 3,
}

_nc_cache = None


def _interleave(primary, filler):
    """Emit primary units with filler units woven in (filler spread evenly)."""
    np_, nf = len(primary), len(filler)
    fi = 0
    for i, u in enumerate(primary):
        u()
        want = int(round((i + 1) * nf / max(np_, 1)))
        while fi < want:
            filler[fi]()
            fi += 1
    while fi < nf:
        filler[fi]()
        fi += 1


def build_bass(dbg=False):
    import concourse.bass as bass
    import concourse.bacc as bacc
    import concourse.tile as tile
    import concourse.mybir as mybir

    F32 = mybir.dt.float32
    F16 = mybir.dt.float16
    Exp = mybir.ActivationFunctionType.Exp
    Mult = mybir.AluOpType.mult
    Add = mybir.AluOpType.add

    nc = bacc.Bacc("TRN2", target_bir_lowering=False, debug=False)

    xt_d = nc.dram_tensor("xt", [C, B * T], F16, kind="ExternalInput")
    wqk_d = nc.dram_tensor("wqk", [128, 2 * KB * 128], F16, kind="ExternalInput")
    wv_d = nc.dram_tensor("wv", [C, HCOLS], F16, kind="ExternalInput")
    bqk_d = nc.dram_tensor("bqk", [HCOLS, 2], F32, kind="ExternalInput")
    bv_d = nc.dram_tensor("bv", [128, 512], F32, kind="ExternalInput")
    ow_d = nc.dram_tensor("ow", [HCOLS, C], F16, kind="ExternalInput")
    tri2_d = nc.dram_tensor("tri2", [128, 256], F16, kind="ExternalInput")
    y_d = nc.dram_tensor("y", [B * T, C], F16, kind="ExternalOutput")

    scale = float(1.0 / np.sqrt(DH))

    with tile.TileContext(nc) as tc:
        with (
            tc.tile_pool(name="const", bufs=1) as constp,
            tc.tile_pool(name="xt", bufs=CFG["xt_bufs"]) as xtp,
            tc.tile_pool(name="qk", bufs=2) as qkp,
            tc.tile_pool(name="vaug", bufs=2) as vaugp,
            tc.tile_pool(name="pt", bufs=1) as ptp,
            tc.tile_pool(name="att", bufs=2) as attp,
            tc.tile_pool(name="recip", bufs=4) as recipp,
            tc.tile_pool(name="yout", bufs=3) as youtp,
            tc.tile_pool(name="ps", bufs=1, space="PSUM") as ps,
        ):
            # ---- constants / weights ----
            wqk_sb = constp.tile([128, KB * 2 * 128], F16)    # [k, (kb, m*128)]
            wv_sb = constp.tile([128, KB * 128], F16)         # [k, (kb, dh2)]
            ow_sb = constp.tile([128, C], F16)
            bqk_sb = constp.tile([HCOLS, 2], F32)
            bv_sb = constp.tile([128, 512], F32)
            tri2_sb = constp.tile([128, 256], F16)

            wqk3 = wqk_sb[:].rearrange("p (m kb d) -> p m kb d", m=2, d=128)
            wv3 = wv_sb[:].rearrange("p (kb d) -> p kb d", d=128)

            # ---- per-batch state (tiles acquired per batch) ----
            st = [dict() for _ in range(B)]   # xt, qT, kT, vaug, pt, attT

            def startup_unit():
                """wqk first, then x^T(b0,H0) in 512-token halves so the first
                qk chunk completes as early as possible."""
                nc.sync.dma_start(wqk_sb[:, 0:1024], wqk_d[:, 0:1024])
                xts = xtp.tile([128, KB * 1024], F16, name="xt00", tag="xt")
                st[0]["xt0"] = xts
                dst = xts[:].rearrange("p (kb t) -> p kb t", t=1024)
                src = xt_d.rearrange("(kb p) t -> p kb t", p=128)[:, :, 0:1024]
                nc.sync.dma_start(dst[:, :, 0:512], src[:, :, 0:512])
                nc.sync.dma_start(wqk_sb[:, 1024:2048], wqk_d[:, 1024:2048])
                nc.sync.dma_start(bqk_sb[:], bqk_d[:])
                nc.sync.dma_start(
                    wv_sb[:].rearrange("p (kb d) -> p kb d", d=128),
                    wv_d.rearrange("(kb p) d -> p kb d", p=128),
                )
                nc.sync.dma_start(bv_sb[:], bv_d[:])
                nc.sync.dma_start(dst[:, :, 512:1024], src[:, :, 512:1024])
                nc.sync.dma_start(ow_sb[:], ow_d[:])
                nc.sync.dma_start(tri2_sb[:], tri2_d[:])

            def xt_dma_unit(b, Hh):
                """DMA x^T for half-batch Hh into a fresh tile."""
                def run():
                    xts = xtp.tile([128, KB * 1024], F16, name=f"xt{b}{Hh}",
                                   tag="xt")
                    st[b][f"xt{Hh}"] = xts
                    dst = xts[:].rearrange("p (kb t) -> p kb t", t=1024)
                    src = xt_d.rearrange("(kb p) t -> p kb t", p=128)[
                        :, :, b * T + Hh * 1024: b * T + Hh * 1024 + 1024]
                    nc.sync.dma_start(dst, src)
                return run

            def alloc_batch_unit(b):
                def run():
                    st[b]["qT"] = qkp.tile([128, T], F16, name=f"qT{b}", tag="qT")
                    st[b]["kT"] = qkp.tile([128, T], F16, name=f"kT{b}", tag="kT")
                    va = vaugp.tile([128, TB * 130], F16, name=f"va{b}", tag="va")
                    st[b]["va"] = va
                    st[b]["pt"] = ptp.tile([128, PT_COLS], F16, name=f"pt{b}",
                                           tag="pt")
                    st[b]["attT"] = attp.tile([128, T], F16, name=f"attT{b}",
                                              tag="attT")
                    # ones columns of vaug (denominator accumulators)
                    ones_view = va[:].rearrange("p (g d) -> p g d", d=65)[:, :, 64:65]
                    nc.vector.memset(ones_view, 1.0)
                return run

            def qk_unit(b, Hh, m, tcl):
                """One 512-token chunk of q^T or k^T (m: 0=q, 1=k)."""
                def run():
                    xts = st[b][f"xt{Hh}"]
                    xv = xts[:].rearrange("p (kb t) -> p kb t", t=1024)
                    pt = ps.tile([128, 512], F32, name="psqk", tag="b1",
                                 bufs=CFG["b1_bufs"])
                    for kb in range(KB):
                        nc.tensor.matmul(
                            pt[:],
                            wqk3[:, m, kb, :],
                            xv[:, kb, tcl * 512:tcl * 512 + 512],
                            start=(kb == 0),
                            stop=(kb == KB - 1),
                        )
                    dstT = st[b]["qT"] if m == 0 else st[b]["kT"]
                    t0 = Hh * 1024 + tcl * 512
                    nc.vector.tensor_scalar_add(
                        dstT[:, t0:t0 + 512], pt[:], bqk_sb[:, m:m + 1])
                return run

            def v_unit(b, Hh, g):
                """v for 4 t-blocks (512 tokens), natural layout into vaug."""
                def run():
                    xts = st[b][f"xt{Hh}"]
                    xv = xts[:].rearrange("p (kb t) -> p kb t", t=1024)
                    va = st[b]["va"]
                    pt = ps.tile([128, 512], F32, name="psv", tag="b1",
                                 bufs=CFG["b1_bufs"])
                    for tbl in range(4):
                        tloc = g * 512 + tbl * 128
                        for kb in range(KB):
                            nc.tensor.matmul(
                                pt[:, tbl * 128:(tbl + 1) * 128],
                                xv[:, kb, tloc:tloc + 128],
                                wv3[:, kb, :],
                                start=(kb == 0),
                                stop=(kb == KB - 1),
                            )
                    # psum [t(128), 4tb x (2h x 64dh)] -> vaug [tb, h*65 .. +64]
                    tb0 = Hh * 8 + g * 4
                    dst = va[:].rearrange("p (tb c) -> p tb c", c=130)[
                        :, tb0:tb0 + 4, :].rearrange("p tb (h c) -> p tb h c",
                                                     h=2)[:, :, :, 0:64]
                    src = pt[:].rearrange("p (tb h c) -> p tb h c", tb=4, h=2)
                    nc.vector.tensor_tensor(
                        dst, src,
                        bv_sb[:].rearrange("p (tb h c) -> p tb h c", tb=4, h=2),
                        Add)
                return run

            def qkv_units(b):
                units = []
                for Hh in range(2):
                    for tcl in range(2):
                        units.append(qk_unit(b, Hh, 0, tcl))
                        units.append(qk_unit(b, Hh, 1, tcl))
                        units.append(v_unit(b, Hh, tcl))
                return units

            # ---- attention ----
            eng_tt = {"dve": nc.vector, "pool": nc.gpsimd}

            def s_exp_part(b, ic, jb):
                """S matmuls + exp (+ diag mask) for (i-chunk ic, j-block jb)."""
                i0 = 512 * ic
                lo = max(0, 128 * jb - i0)          # >0 only on diagonal chunk
                ext = EXTS[jb]
                off = OFFS[jb]
                rel = i0 + lo - 128 * jb            # chunk start within extent
                n = 512 - lo

                def run():
                    qT, kT = st[b]["qT"], st[b]["kT"]
                    ptt = st[b]["pt"]
                    sc = ps.tile([128, 1024], F32, name="pssc", tag="sq",
                                 bufs=CFG["sq_bufs"])
                    for h in range(HL):
                        nc.tensor.matmul(
                            sc[:, h * 512 + lo:(h + 1) * 512],
                            kT[h * 64:(h + 1) * 64, 128 * jb:128 * jb + 128],
                            qT[h * 64:(h + 1) * 64, i0 + lo:i0 + 512],
                            start=True, stop=True,
                        )
                    # exp -> P^T arena (2-segment strided AP covers both heads)
                    src = sc[:].rearrange("p (h x) -> p h x", h=2)[:, :, lo:512]
                    dst = ptt[:, off:off + 2 * ext].rearrange(
                        "p (h x) -> p h x", h=2)[:, :, rel:rel + n]
                    nc.scalar.activation(dst, src, Exp, scale=scale)
                    # mask the diagonal block right after its exp
                    if rel == 0:
                        mdst = ptt[:, off:off + 2 * ext].rearrange(
                            "p (h x) -> p h x", h=2)[:, :, 0:128]
                        meng = eng_tt[CFG["mask_eng"][jb % len(CFG["mask_eng"])]]
                        meng.tensor_tensor(
                            mdst, mdst,
                            tri2_sb[:].rearrange("p (h x) -> p h x", h=2),
                            Mult)
                return run

            def pv_part(b, ic, jb):
                i0 = 512 * ic
                lo = max(0, 128 * jb - i0)
                ext = EXTS[jb]
                off = OFFS[jb]
                rel = i0 + lo - 128 * jb
                n = 512 - lo
                n_jb = 4 * (ic + 1)

                def run():
                    ptt = st[b]["pt"]
                    va = st[b]["va"]
                    if jb == 0:
                        for h in range(HL):
                            st[b][f"pv{h}"] = ps.tile(
                                [128, 512], F32, name=f"pspv{h}", tag="pv",
                                bufs=CFG["pv_bufs"])
                    for h in range(HL):
                        nc.tensor.matmul(
                            st[b][f"pv{h}"][0:65, lo:512],
                            va[:, jb * 130 + h * 65:jb * 130 + h * 65 + 65],
                            ptt[:, off + h * ext + rel:off + h * ext + rel + n],
                            start=(jb == 0),
                            stop=(jb == n_jb - 1),
                        )
                return run

            def norm_part(b, ic, tail=False):
                i0 = 512 * ic

                def run():
                    attT = st[b]["attT"]
                    for h in range(HL):
                        pv = st[b][f"pv{h}"]
                        rrow = recipp.tile([1, 512], F32, name="rrow", tag="rr")
                        rbc = recipp.tile([64, 512], F32, name="rbc", tag="rb")
                        if tail:
                            continue   # handled jointly below (g-outer)
                        # free the pv psum bank early for the next chunk:
                        # stage to SBUF on DVE (b0) or ACT (b1, DVE is busier)
                        au = recipp.tile([65, 512], F32, name="attU",
                                         tag="au", bufs=2)
                        if b == 0:
                            nc.vector.tensor_copy(au[:], pv[0:65, :])
                        else:
                            nc.scalar.copy(au[:], pv[0:65, :])
                        nc.vector.reciprocal(rrow[:], au[64:65, :])
                        nc.gpsimd.partition_broadcast(rbc[:], rrow[:])
                        nc.vector.tensor_tensor(
                            attT[h * 64:(h + 1) * 64, i0:i0 + 512],
                            au[0:64, :], rbc[:], Mult)
                    if tail:
                        # latency-critical: per-128-col blocks, g-outer so both
                        # heads of each t-block finish together; stage psum to
                        # SBUF on ACT so DVE/Pool/ACT pipeline in parallel
                        aus = [recipp.tile([65, 512], F32, name="attU",
                                           tag="au", bufs=2) for _ in range(HL)]
                        rrows = [recipp.tile([1, 512], F32, name="rrow",
                                             tag="rr") for _ in range(HL)]
                        rbcs = [recipp.tile([64, 512], F32, name="rbc",
                                            tag="rb") for _ in range(HL)]
                        for g in range(4):
                            gs = slice(g * 128, (g + 1) * 128)
                            for h in range(HL):
                                pv = st[b][f"pv{h}"]
                                nc.scalar.copy(aus[h][:, gs], pv[0:65, gs])
                                nc.vector.reciprocal(rrows[h][:, gs],
                                                     aus[h][64:65, gs])
                                nc.gpsimd.partition_broadcast(rbcs[h][:, gs],
                                                              rrows[h][:, gs])
                                nc.vector.tensor_tensor(
                                    attT[h * 64:(h + 1) * 64,
                                         i0 + g * 128:i0 + (g + 1) * 128],
                                    aus[h][0:64, gs], rbcs[h][:, gs], Mult)
                return run

            def attention_units(b, after_chunk=None):
                """Software-pipelined: unit k = S/exp(order[k]) + PV(order[k-1]).
                after_chunk[ic] (optional) -> list of units spliced right after
                the unit that closes chunk ic (PV last + norm).
                Returns the unit list."""
                order = [(ic, jb) for ic in range(NCH)
                         for jb in range(4 * (ic + 1))]
                units = []

                def mk(parts):
                    def run():
                        for p in parts:
                            p()
                    return run

                prev = None
                pending = []
                for (ic, jb) in order:
                    parts = [s_exp_part(b, ic, jb)]
                    if prev is not None:
                        parts.append(pv_part(b, *prev))
                        if prev[1] == 4 * (prev[0] + 1) - 1:
                            parts.append(norm_part(b, prev[0], tail=(b == 1 and prev[0] == NCH - 1)))
                    units.append(mk(parts))
                    if pending:
                        units.extend(pending)
                        pending = []
                    if prev is not None and prev[1] == 4 * (prev[0] + 1) - 1 \
                            and after_chunk:
                        pending = list(after_chunk.get(prev[0], []))
                    prev = (ic, jb)
                if pending:
                    units.extend(pending)
                    pending = []
                units.append(mk([pv_part(b, *prev), norm_part(b, prev[0], tail=(b == 1 and prev[0] == NCH - 1))]))
                if after_chunk:
                    units.extend(after_chunk.get(NCH - 1, []))
                return units

            # ---- output projection ----
            def oproj_unit(b, tb):
                def run():
                    attT = st[b]["attT"]
                    yo = youtp.tile([128, C], F16, name="yo", tag="yo")
                    for ec in range(2):
                        pt = ps.tile([128, 512], F32, name="psy", tag="b1",
                                     bufs=CFG["b1_bufs"])
                        nc.tensor.matmul(
                            pt[:],
                            attT[:, tb * 128:(tb + 1) * 128],
                            ow_sb[:, ec * 512:(ec + 1) * 512],
                            start=True, stop=True,
                        )
                        if b == 1 and tb >= 12:
                            # tail: split copies across ACT+DVE so the last
                            # blocks drain in parallel
                            eng = "act" if ec == 0 else "dve"
                        else:
                            eng = CFG["yo_eng"][(b * TB + tb * 2 + ec)
                                                % len(CFG["yo_eng"])]
                        dst = yo[:, ec * 512:(ec + 1) * 512]
                        if eng == "dve":
                            nc.vector.tensor_copy(dst, pt[:])
                        elif eng == "act":
                            nc.scalar.copy(dst, pt[:])
                        else:
                            nc.gpsimd.tensor_copy(dst, pt[:])
                        nc.sync.dma_start(
                            y_d[b * T + tb * 128:b * T + (tb + 1) * 128,
                                ec * 512:(ec + 1) * 512], dst)
                return run

            def oproj_units(b):
                return [oproj_unit(b, tb) for tb in range(TB)]

            # ---- schedule ----
            startup_unit()
            alloc_batch_unit(0)()
            qk0 = qkv_units(0)
            a0 = attention_units(0)      # 41 pipelined units
            # b0: attention chunks woven into qkv as soon as deps exist
            seq0 = (qk0[0:3] + a0[0:4] + [xt_dma_unit(0, 1)] + qk0[3:6]
                    + a0[4:12] + qk0[6:9] + a0[12:24] + qk0[9:12] + a0[24:41])
            q1 = qkv_units(1)
            fill1 = ([xt_dma_unit(1, 0), alloc_batch_unit(1)] + q1[0:6]
                     + [xt_dma_unit(1, 1)] + q1[6:12])
            # keep early PE stream unobstructed; weave b1 qkv into the tail
            for u in seq0[:12]:
                u()
            _interleave(seq0[12:], fill1)
            o0 = oproj_units(0)
            o1 = oproj_units(1)
            a1 = attention_units(1, after_chunk={
                0: o1[0:4], 1: o1[4:8], 2: o1[8:12], 3: o1[12:16]})
            _interleave(a1, o0)

    nc.compile()
    return nc


def _prep_inputs(x, qkv_w, qkv_b, o_w):
    """Per-core input maps (head sharding), with host-side transpose/casts."""
    x = np.asarray(x, dtype=np.float32)
    qkv_w = np.asarray(qkv_w, dtype=np.float32)
    qkv_b = np.asarray(qkv_b, dtype=np.float32)
    o_w = np.asarray(o_w, dtype=np.float32)

    xt = np.ascontiguousarray(
        x.reshape(B * T, C).T.astype(np.float16))          # [C, B*T]
    tri = np.triu(np.ones((128, 128), dtype=np.float16))
    tri2 = np.ascontiguousarray(np.concatenate([tri, tri], axis=1))

    in_maps = []
    for c in range(NCORES):
        lo = c * HCOLS
        wq = qkv_w[:, lo:lo + HCOLS]
        wk = qkv_w[:, C + lo:C + lo + HCOLS]
        wv = qkv_w[:, 2 * C + lo:2 * C + lo + HCOLS]
        bq = qkv_b[lo:lo + HCOLS]
        bk = qkv_b[C + lo:C + lo + HCOLS]
        bv = qkv_b[2 * C + lo:2 * C + lo + HCOLS]
        # v bias replicated for the 4-t-block psum layout [tb(4) x dh2(128)]
        bv_rep = np.tile(bv[None, :], (128, 4)).astype(np.float32)
        in_maps.append({
            "xt": xt,
            "wqk": np.ascontiguousarray(np.concatenate(
                [w.reshape(8, 128, 128).transpose(1, 0, 2).reshape(128, 1024)
                 for w in (wq, wk)], axis=1).astype(np.float16)),
            "wv": np.ascontiguousarray(wv.astype(np.float16)),
            "bqk": np.ascontiguousarray(
                np.stack([bq, bk], axis=1).astype(np.float32)),
            "bv": np.ascontiguousarray(bv_rep),
            "ow": np.ascontiguousarray(
                o_w[lo:lo + HCOLS, :].astype(np.float16)),
            "tri2": tri2,
        })
    return in_maps


def kernel(x, qkv_w, qkv_b, o_w, o_b):
    global _nc_cache
    from concourse import bass_utils
    if _nc_cache is None:
        _nc_cache = build_bass()
    nc = _nc_cache
    in_maps = _prep_inputs(x, qkv_w, qkv_b, o_w)
    res = bass_utils.run_bass_kernel_spmd(nc, in_maps, core_ids=list(range(NCORES)))
    y = np.zeros((B * T, C), dtype=np.float64)
    for c in range(NCORES):
        y += res.results[c]["y"].astype(np.float64)
    y = (y + np.asarray(o_b, dtype=np.float64)[None, :]).astype(np.float32)
    return y.reshape(B, T, C)


# revision 5
# speedup vs baseline: 1.0386x; 1.0069x over previous
"""Causal self-attention (B=2, T=2048, C=1024, H=16) on 8 TRN2 NeuronCores.

Sharding: tensor-parallel over heads - core c owns heads {2c, 2c+1} for both
batches (qkv_w column slice, o_w row slice). Each core computes a partial
o_proj output; the host sums the 8 partials (fp64) and adds o_b.

Design (all fp16 operands, fp32 psum accumulation):
  - x^T is pre-transposed ON HOST and fed as an fp16 input: no PE transposes,
    no psum->sbuf x copies, half the DMA bytes. Weights/biases are pre-sliced
    and cast on host as well.
  - q^T/k^T = W^T x^T (w stationary, xT moving 512 wide); the bias add rides
    the mandatory psum->sbuf copy (DVE tensor_scalar).
  - v is produced in NATURAL [t, dh] layout (stationary xT block, moving w_v)
    directly into vaug tiles carrying a ones column (softmax denominator
    accumulates inside the PV matmul for free).
  - S^T[j,i] per (batch, j-block, i-chunk) with causal trimming; one fused
    exp per chunk on ACT (scale folded in, both heads via a 2-segment AP)
    into a per-batch P^T arena; diagonal blocks masked by a 0/1 triangle
    on GpSimd (sbuf-only).
  - PV: vaug stationary [j,65], P^T moving -> unnormalized attT psum
    [65, 512]; row 64 is the denominator. reciprocal (DVE) + partition
    broadcast (GpSimd) + normalize-multiply into attT fp16.
  - oproj: attT stationary, ow moving; psum->sbuf copies balanced across
    DVE/ACT; y partials stream out per 512-column half as fp16.
  Schedule: software-pipelined attention units (PV lags S/exp by one unit so
  the ACT exp latency hides behind the next S matmuls); attention(b0) starts
  during qkv(b0); qkv(b1) fills attention(b0); oproj(b0) fills
  attention(b1); oproj(b1) chunks are spliced in as their attT columns
  normalize, with an ACT/DVE-parallel drain for the last blocks.
"""

import numpy as np

B = 2
T = 2048
C = 1024
H = 16
DH = 64
NCORES = 8
HL = 2                      # heads per core
HCOLS = HL * DH             # 128
KB = C // 128               # 8 contraction blocks
TB = T // 128               # 16 t-blocks per batch
NCH = T // 512              # 4 i-chunks per batch

# per-jb P^T extents (columns per head) and offsets in the per-batch arena
EXTS = [T - 128 * jb for jb in range(TB)]
OFFS = np.cumsum([0] + [2 * e for e in EXTS]).tolist()
PT_COLS = OFFS[-1]          # 2 * 17408 = 34816

CFG = {
    "norm_eng": ["dve"],                  # rotation for normalize-mult (psum: no pool)
    "yo_eng": ["dve"],
    "mask_eng": ["pool"],                 # P^T masks are sbuf-only: pool OK
    "sq_bufs": 2,
    "pv_bufs": 2,
    "b1_bufs": 2,
    "xt_bufs": 3,
}

_nc_cache = None


def _interleave(primary, filler):
    """Emit primary units with filler units woven in (filler spread evenly)."""
    np_, nf = len(primary), len(filler)
    fi = 0
    for i, u in enumerate(primary):
        u()
        want = int(round((i + 1) * nf / max(np_, 1)))
        while fi < want:
            filler[fi]()
            fi += 1
    while fi < nf:
        filler[fi]()
        fi += 1


def build_bass(dbg=False):
    import concourse.bass as bass
    import concourse.bacc as bacc
    import concourse.tile as tile
    import concourse.mybir as mybir

    F32 = mybir.dt.float32
    F16 = mybir.dt.float16
    Exp = mybir.ActivationFunctionType.Exp
    Mult = mybir.AluOpType.mult
    Add = mybir.AluOpType.add

    nc = bacc.Bacc("TRN2", target_bir_lowering=False, debug=False)

    xt_d = nc.dram_tensor("xt", [C, B * T], F16, kind="ExternalInput")
    wqk_d = nc.dram_tensor("wqk", [128, 2 * KB * 128], F16, kind="ExternalInput")
    wv_d = nc.dram_tensor("wv", [128, KB * 128], F16, kind="ExternalInput")
    bqk_d = nc.dram_tensor("bqk", [HCOLS, 2], F32, kind="ExternalInput")
    bv_d = nc.dram_tensor("bv", [128, 512], F32, kind="ExternalInput")
    ow_d = nc.dram_tensor("ow", [HCOLS, C], F16, kind="ExternalInput")
    tri2_d = nc.dram_tensor("tri2", [128, 256], F16, kind="ExternalInput")
    y_d = nc.dram_tensor("y", [B * T, C], F16, kind="ExternalOutput")

    scale = float(1.0 / np.sqrt(DH))

    with tile.TileContext(nc) as tc:
        with (
            tc.tile_pool(name="const", bufs=1) as constp,
            tc.tile_pool(name="xt", bufs=CFG["xt_bufs"]) as xtp,
            tc.tile_pool(name="qk", bufs=2) as qkp,
            tc.tile_pool(name="vaug", bufs=2) as vaugp,
            tc.tile_pool(name="pt", bufs=1) as ptp,
            tc.tile_pool(name="att", bufs=2) as attp,
            tc.tile_pool(name="recip", bufs=4) as recipp,
            tc.tile_pool(name="yout", bufs=3) as youtp,
            tc.tile_pool(name="ps", bufs=1, space="PSUM") as ps,
        ):
            # ---- constants / weights ----
            wqk_sb = constp.tile([128, KB * 2 * 128], F16)    # [k, (kb, m*128)]
            wv_sb = constp.tile([128, KB * 128], F16)         # [k, (kb, dh2)]
            ow_sb = constp.tile([128, C], F16)
            bqk_sb = constp.tile([HCOLS, 2], F32)
            bv_sb = constp.tile([128, 512], F32)
            tri2_sb = constp.tile([128, 256], F16)

            wqk3 = wqk_sb[:].rearrange("p (m kb d) -> p m kb d", m=2, d=128)
            wv3 = wv_sb[:].rearrange("p (kb d) -> p kb d", d=128)

            # ---- per-batch state (tiles acquired per batch) ----
            st = [dict() for _ in range(B)]   # xt, qT, kT, vaug, pt, attT

            def startup_unit():
                """wqk first, then x^T(b0,H0) in 512-token halves so the first
                qk chunk completes as early as possible."""
                nc.sync.dma_start(wqk_sb[:, 0:1024], wqk_d[:, 0:1024])
                xts = xtp.tile([128, KB * 1024], F16, name="xt00", tag="xt")
                st[0]["xt0"] = xts
                dst = xts[:].rearrange("p (kb t) -> p kb t", t=1024)
                src = xt_d.rearrange("(kb p) t -> p kb t", p=128)[:, :, 0:1024]
                nc.sync.dma_start(dst[:, :, 0:512], src[:, :, 0:512])
                nc.sync.dma_start(wqk_sb[:, 1024:2048], wqk_d[:, 1024:2048])
                nc.sync.dma_start(bqk_sb[:], bqk_d[:])
                nc.sync.dma_start(wv_sb[:], wv_d[:])
                nc.sync.dma_start(bv_sb[:], bv_d[:])
                nc.sync.dma_start(dst[:, :, 512:1024], src[:, :, 512:1024])
                nc.sync.dma_start(ow_sb[:], ow_d[:])
                nc.sync.dma_start(tri2_sb[:], tri2_d[:])

            def xt_dma_unit(b, Hh):
                """DMA x^T for half-batch Hh into a fresh tile."""
                def run():
                    xts = xtp.tile([128, KB * 1024], F16, name=f"xt{b}{Hh}",
                                   tag="xt")
                    st[b][f"xt{Hh}"] = xts
                    dst = xts[:].rearrange("p (kb t) -> p kb t", t=1024)
                    src = xt_d.rearrange("(kb p) t -> p kb t", p=128)[
                        :, :, b * T + Hh * 1024: b * T + Hh * 1024 + 1024]
                    nc.sync.dma_start(dst, src)
                return run

            def alloc_batch_unit(b):
                def run():
                    st[b]["qT"] = qkp.tile([128, T], F16, name=f"qT{b}", tag="qT")
                    st[b]["kT"] = qkp.tile([128, T], F16, name=f"kT{b}", tag="kT")
                    va = vaugp.tile([128, TB * 130], F16, name=f"va{b}", tag="va")
                    st[b]["va"] = va
                    st[b]["pt"] = ptp.tile([128, PT_COLS], F16, name=f"pt{b}",
                                           tag="pt")
                    st[b]["attT"] = attp.tile([128, T], F16, name=f"attT{b}",
                                              tag="attT")
                    # ones columns of vaug (denominator accumulators)
                    ones_view = va[:].rearrange("p (g d) -> p g d", d=65)[:, :, 64:65]
                    nc.vector.memset(ones_view, 1.0)
                return run

            def qk_unit(b, Hh, m, tcl):
                """One 512-token chunk of q^T or k^T (m: 0=q, 1=k)."""
                def run():
                    xts = st[b][f"xt{Hh}"]
                    xv = xts[:].rearrange("p (kb t) -> p kb t", t=1024)
                    pt = ps.tile([128, 512], F32, name="psqk", tag="b1",
                                 bufs=CFG["b1_bufs"])
                    for kb in range(KB):
                        nc.tensor.matmul(
                            pt[:],
                            wqk3[:, m, kb, :],
                            xv[:, kb, tcl * 512:tcl * 512 + 512],
                            start=(kb == 0),
                            stop=(kb == KB - 1),
                        )
                    dstT = st[b]["qT"] if m == 0 else st[b]["kT"]
                    t0 = Hh * 1024 + tcl * 512
                    nc.vector.tensor_scalar_add(
                        dstT[:, t0:t0 + 512], pt[:], bqk_sb[:, m:m + 1])
                return run

            def v_unit(b, Hh, g):
                """v for 4 t-blocks (512 tokens), natural layout into vaug."""
                def run():
                    xts = st[b][f"xt{Hh}"]
                    xv = xts[:].rearrange("p (kb t) -> p kb t", t=1024)
                    va = st[b]["va"]
                    pt = ps.tile([128, 512], F32, name="psv", tag="b1",
                                 bufs=CFG["b1_bufs"])
                    for tbl in range(4):
                        tloc = g * 512 + tbl * 128
                        for kb in range(KB):
                            nc.tensor.matmul(
                                pt[:, tbl * 128:(tbl + 1) * 128],
                                xv[:, kb, tloc:tloc + 128],
                                wv3[:, kb, :],
                                start=(kb == 0),
                                stop=(kb == KB - 1),
                            )
                    # psum [t(128), 4tb x (2h x 64dh)] -> vaug [tb, h*65 .. +64]
                    tb0 = Hh * 8 + g * 4
                    dst = va[:].rearrange("p (tb c) -> p tb c", c=130)[
                        :, tb0:tb0 + 4, :].rearrange("p tb (h c) -> p tb h c",
                                                     h=2)[:, :, :, 0:64]
                    src = pt[:].rearrange("p (tb h c) -> p tb h c", tb=4, h=2)
                    nc.vector.tensor_tensor(
                        dst, src,
                        bv_sb[:].rearrange("p (tb h c) -> p tb h c", tb=4, h=2),
                        Add)
                return run

            def qkv_units(b):
                units = []
                for Hh in range(2):
                    for tcl in range(2):
                        units.append(qk_unit(b, Hh, 0, tcl))
                        units.append(qk_unit(b, Hh, 1, tcl))
                        units.append(v_unit(b, Hh, tcl))
                return units

            # ---- attention ----
            eng_tt = {"dve": nc.vector, "pool": nc.gpsimd}

            def s_exp_part(b, ic, jb):
                """S matmuls + exp (+ diag mask) for (i-chunk ic, j-block jb)."""
                i0 = 512 * ic
                lo = max(0, 128 * jb - i0)          # >0 only on diagonal chunk
                ext = EXTS[jb]
                off = OFFS[jb]
                rel = i0 + lo - 128 * jb            # chunk start within extent
                n = 512 - lo

                def run():
                    qT, kT = st[b]["qT"], st[b]["kT"]
                    ptt = st[b]["pt"]
                    sc = ps.tile([128, 1024], F32, name="pssc", tag="sq",
                                 bufs=CFG["sq_bufs"])
                    for h in range(HL):
                        nc.tensor.matmul(
                            sc[:, h * 512 + lo:(h + 1) * 512],
                            kT[h * 64:(h + 1) * 64, 128 * jb:128 * jb + 128],
                            qT[h * 64:(h + 1) * 64, i0 + lo:i0 + 512],
                            start=True, stop=True,
                        )
                    # exp -> P^T arena (2-segment strided AP covers both heads)
                    src = sc[:].rearrange("p (h x) -> p h x", h=2)[:, :, lo:512]
                    dst = ptt[:, off:off + 2 * ext].rearrange(
                        "p (h x) -> p h x", h=2)[:, :, rel:rel + n]
                    nc.scalar.activation(dst, src, Exp, scale=scale)
                    # mask the diagonal block right after its exp
                    if rel == 0:
                        mdst = ptt[:, off:off + 2 * ext].rearrange(
                            "p (h x) -> p h x", h=2)[:, :, 0:128]
                        meng = eng_tt[CFG["mask_eng"][jb % len(CFG["mask_eng"])]]
                        meng.tensor_tensor(
                            mdst, mdst,
                            tri2_sb[:].rearrange("p (h x) -> p h x", h=2),
                            Mult)
                return run

            def pv_part(b, ic, jb):
                i0 = 512 * ic
                lo = max(0, 128 * jb - i0)
                ext = EXTS[jb]
                off = OFFS[jb]
                rel = i0 + lo - 128 * jb
                n = 512 - lo
                n_jb = 4 * (ic + 1)

                def run():
                    ptt = st[b]["pt"]
                    va = st[b]["va"]
                    if jb == 0:
                        for h in range(HL):
                            st[b][f"pv{h}"] = ps.tile(
                                [128, 512], F32, name=f"pspv{h}", tag="pv",
                                bufs=CFG["pv_bufs"])
                    for h in range(HL):
                        nc.tensor.matmul(
                            st[b][f"pv{h}"][0:65, lo:512],
                            va[:, jb * 130 + h * 65:jb * 130 + h * 65 + 65],
                            ptt[:, off + h * ext + rel:off + h * ext + rel + n],
                            start=(jb == 0),
                            stop=(jb == n_jb - 1),
                        )
                return run

            def norm_part(b, ic, tail=False):
                i0 = 512 * ic

                def run():
                    attT = st[b]["attT"]
                    for h in range(HL):
                        pv = st[b][f"pv{h}"]
                        rrow = recipp.tile([1, 512], F32, name="rrow", tag="rr")
                        rbc = recipp.tile([64, 512], F32, name="rbc", tag="rb")
                        if tail:
                            continue   # handled jointly below (g-outer)
                        # free the pv psum bank early for the next chunk:
                        # stage to SBUF on DVE (b0) or ACT (b1, DVE is busier)
                        au = recipp.tile([65, 512], F32, name="attU",
                                         tag="au", bufs=2)
                        if b == 0:
                            nc.vector.tensor_copy(au[:], pv[0:65, :])
                        else:
                            nc.scalar.copy(au[:], pv[0:65, :])
                        nc.vector.reciprocal(rrow[:], au[64:65, :])
                        nc.gpsimd.partition_broadcast(rbc[:], rrow[:])
                        nc.vector.tensor_tensor(
                            attT[h * 64:(h + 1) * 64, i0:i0 + 512],
                            au[0:64, :], rbc[:], Mult)
                    if tail:
                        # latency-critical: per-128-col blocks, g-outer so both
                        # heads of each t-block finish together; stage psum to
                        # SBUF on ACT so DVE/Pool/ACT pipeline in parallel
                        aus = [recipp.tile([65, 512], F32, name="attU",
                                           tag="au", bufs=2) for _ in range(HL)]
                        rrows = [recipp.tile([1, 512], F32, name="rrow",
                                             tag="rr") for _ in range(HL)]
                        rbcs = [recipp.tile([64, 512], F32, name="rbc",
                                            tag="rb") for _ in range(HL)]
                        for g in range(4):
                            gs = slice(g * 128, (g + 1) * 128)
                            for h in range(HL):
                                pv = st[b][f"pv{h}"]
                                nc.scalar.copy(aus[h][:, gs], pv[0:65, gs])
                                nc.vector.reciprocal(rrows[h][:, gs],
                                                     aus[h][64:65, gs])
                                nc.gpsimd.partition_broadcast(rbcs[h][:, gs],
                                                              rrows[h][:, gs])
                                nc.vector.tensor_tensor(
                                    attT[h * 64:(h + 1) * 64,
                                         i0 + g * 128:i0 + (g + 1) * 128],
                                    aus[h][0:64, gs], rbcs[h][:, gs], Mult)
                return run

            def attention_units(b, after_chunk=None):
                """Software-pipelined: unit k = S/exp(order[k]) + PV(order[k-1]).
                after_chunk[ic] (optional) -> list of units spliced right after
                the unit that closes chunk ic (PV last + norm).
                Returns the unit list."""
                order = [(ic, jb) for ic in range(NCH)
                         for jb in range(4 * (ic + 1))]
                units = []

                def mk(parts):
                    def run():
                        for p in parts:
                            p()
                    return run

                prev = None
                pending = []
                for (ic, jb) in order:
                    parts = [s_exp_part(b, ic, jb)]
                    if prev is not None:
                        parts.append(pv_part(b, *prev))
                        if prev[1] == 4 * (prev[0] + 1) - 1:
                            parts.append(norm_part(b, prev[0], tail=(b == 1 and prev[0] == NCH - 1)))
                    units.append(mk(parts))
                    if pending:
                        units.extend(pending)
                        pending = []
                    if prev is not None and prev[1] == 4 * (prev[0] + 1) - 1 \
                            and after_chunk:
                        pending = list(after_chunk.get(prev[0], []))
                    prev = (ic, jb)
                if pending:
                    units.extend(pending)
                    pending = []
                units.append(mk([pv_part(b, *prev), norm_part(b, prev[0], tail=(b == 1 and prev[0] == NCH - 1))]))
                if after_chunk:
                    units.extend(after_chunk.get(NCH - 1, []))
                return units

            # ---- output projection ----
            def oproj_unit(b, tb):
                def run():
                    attT = st[b]["attT"]
                    yo = youtp.tile([128, C], F16, name="yo", tag="yo")
                    for ec in range(2):
                        pt = ps.tile([128, 512], F32, name="psy", tag="b1",
                                     bufs=CFG["b1_bufs"])
                        nc.tensor.matmul(
                            pt[:],
                            attT[:, tb * 128:(tb + 1) * 128],
                            ow_sb[:, ec * 512:(ec + 1) * 512],
                            start=True, stop=True,
                        )
                        if b == 1 and tb >= 12:
                            # tail: split copies across ACT+DVE so the last
                            # blocks drain in parallel
                            eng = "act" if ec == 0 else "dve"
                        else:
                            eng = CFG["yo_eng"][(b * TB + tb * 2 + ec)
                                                % len(CFG["yo_eng"])]
                        dst = yo[:, ec * 512:(ec + 1) * 512]
                        if eng == "dve":
                            nc.vector.tensor_copy(dst, pt[:])
                        elif eng == "act":
                            nc.scalar.copy(dst, pt[:])
                        else:
                            nc.gpsimd.tensor_copy(dst, pt[:])
                        nc.sync.dma_start(
                            y_d[b * T + tb * 128:b * T + (tb + 1) * 128,
                                ec * 512:(ec + 1) * 512], dst)
                return run

            def oproj_units(b):
                return [oproj_unit(b, tb) for tb in range(TB)]

            # ---- schedule ----
            startup_unit()
            alloc_batch_unit(0)()
            qk0 = qkv_units(0)
            a0 = attention_units(0)      # 41 pipelined units
            # b0: attention chunks woven into qkv as soon as deps exist
            seq0 = (qk0[0:3] + a0[0:4] + [xt_dma_unit(0, 1)] + qk0[3:6]
                    + a0[4:12] + qk0[6:9] + a0[12:24] + qk0[9:12] + a0[24:41])
            q1 = qkv_units(1)
            fill1 = ([xt_dma_unit(1, 0), alloc_batch_unit(1)] + q1[0:6]
                     + [xt_dma_unit(1, 1)] + q1[6:12])
            # keep early PE stream unobstructed; weave b1 qkv into the tail
            for u in seq0[:12]:
                u()
            _interleave(seq0[12:], fill1)
            o0 = oproj_units(0)
            o1 = oproj_units(1)
            a1 = attention_units(1, after_chunk={
                0: o1[0:4], 1: o1[4:8], 2: o1[8:12], 3: o1[12:16]})
            _interleave(a1, o0)

    nc.compile()
    return nc


def _prep_inputs(x, qkv_w, qkv_b, o_w):
    """Per-core input maps (head sharding), with host-side transpose/casts."""
    x = np.asarray(x, dtype=np.float32)
    qkv_w = np.asarray(qkv_w, dtype=np.float32)
    qkv_b = np.asarray(qkv_b, dtype=np.float32)
    o_w = np.asarray(o_w, dtype=np.float32)

    xt = np.ascontiguousarray(
        x.reshape(B * T, C).T.astype(np.float16))          # [C, B*T]
    tri = np.triu(np.ones((128, 128), dtype=np.float16))
    tri2 = np.ascontiguousarray(np.concatenate([tri, tri], axis=1))

    in_maps = []
    for c in range(NCORES):
        lo = c * HCOLS
        wq = qkv_w[:, lo:lo + HCOLS]
        wk = qkv_w[:, C + lo:C + lo + HCOLS]
        wv = qkv_w[:, 2 * C + lo:2 * C + lo + HCOLS]
        bq = qkv_b[lo:lo + HCOLS]
        bk = qkv_b[C + lo:C + lo + HCOLS]
        bv = qkv_b[2 * C + lo:2 * C + lo + HCOLS]
        # v bias replicated for the 4-t-block psum layout [tb(4) x dh2(128)]
        bv_rep = np.tile(bv[None, :], (128, 4)).astype(np.float32)
        in_maps.append({
            "xt": xt,
            "wqk": np.ascontiguousarray(np.concatenate(
                [w.reshape(8, 128, 128).transpose(1, 0, 2).reshape(128, 1024)
                 for w in (wq, wk)], axis=1).astype(np.float16)),
            "wv": np.ascontiguousarray(
                wv.reshape(8, 128, 128).transpose(1, 0, 2)
                .reshape(128, 1024).astype(np.float16)),
            "bqk": np.ascontiguousarray(
                np.stack([bq, bk], axis=1).astype(np.float32)),
            "bv": np.ascontiguousarray(bv_rep),
            "ow": np.ascontiguousarray(
                o_w[lo:lo + HCOLS, :].astype(np.float16)),
            "tri2": tri2,
        })
    return in_maps


def kernel(x, qkv_w, qkv_b, o_w, o_b):
    global _nc_cache
    from concourse import bass_utils
    if _nc_cache is None:
        _nc_cache = build_bass()
    nc = _nc_cache
    in_maps = _prep_inputs(x, qkv_w, qkv_b, o_w)
    res = bass_utils.run_bass_kernel_spmd(nc, in_maps, core_ids=list(range(NCORES)))
    y = np.zeros((B * T, C), dtype=np.float64)
    for c in range(NCORES):
        y += res.results[c]["y"].astype(np.float64)
    y = (y + np.asarray(o_b, dtype=np.float64)[None, :]).astype(np.float32)
    return y.reshape(B, T, C)
